# revision 16
# baseline (speedup 1.0000x reference)
"""Trainium2 Bass kernel for nn_PowerAmpStage.

Pipeline per core (1 batch row per core, chunk-major layout [128 part x 1024]):
  1. envelope follower  : |x| one-pole IIR -> two hardware tensor_tensor_scans
                          (zero-init pass for chunk-end states, then exact pass
                          with per-chunk initial states; c^1024 ~ 2e-22 so the
                          chunk-end states of the zero-init pass are exact)
  2. sag gain           : y1 = x * (1 - sag_depth * env)
  3. pre biquad cascade : truncated impulse response (256 taps) as PE matmuls
                          in a block-major layout obtained via PE transposes
  4. GRU (1 unit)       : K Jacobi sweeps of the full nonlinear recurrence with
                          lagged chunk-boundary states (the GRU forgets at
                          ~0.1/step, so each sweep contracts the error ~10x)
  5. post cascade + knob-controlled feedback biquad: single combined truncated
                          impulse response (256 taps, per-row) as PE matmuls
All parameter preprocessing (stable-biquad map, tiny MLPs, impulse responses,
Toeplitz tables) is O(few thousand) flops on the host.
"""

import numpy as np

import concourse.bacc as bacc
import concourse.bass as bass
import concourse.tile as tile
from concourse import mybir
from concourse.bass_utils import run_bass_kernel_spmd

F32 = mybir.dt.float32
AF = mybir.ActivationFunctionType
OP = mybir.AluOpType

B, L = 8, 131072
NP, CH = 128, 1024
KLEN = 256          # truncated FIR length for both conv stages
NSHIFT = KLEN // NP  # 2
NCONST = 14
INF = 2 * CH + 16 + 2 * NSHIFT * NP + NP  # 2704: x | consts(16) | crow | tpre | tpost | ident
K_SWEEPS = 8

_CACHE = {}


# ---------------- host-side parameter preprocessing (tiny) ----------------

def _sigmoid(v):
    return 1.0 / (1.0 + np.exp(-v))


def _stable(raw):
    b = raw[..., :3]
    a1 = 2.0 * np.tanh(raw[..., 3])
    a2 = ((2.0 - np.abs(a1)) * np.tanh(raw[..., 4]) + np.abs(a1)) / 2.0
    return np.concatenate([b, a1[..., None], a2[..., None]], axis=-1)


def _biquad_ir(c, n):
    # impulse response of DF2T biquad [b0,b1,b2,a1,a2]
    y = np.zeros(n)
    s1 = 0.0
    s2 = 0.0
    for t in range(n):
        xt = 1.0 if t == 0 else 0.0
        yt = c[0] * xt + s1
        s1n = c[1] * xt - c[3] * yt + s2
        s2 = c[2] * xt - c[4] * yt
        s1 = s1n
        y[t] = yt
    return y


def _toeplitz(kern):
    # T[k, s*128 + i] = kern[i - k + 128*s]
    t = np.zeros((NP, NSHIFT * NP), dtype=np.float32)
    k_idx = np.arange(NP)[:, None]
    for s in range(NSHIFT):
        i_idx = np.arange(NP)[None, :]
        j = i_idx - k_idx + NP * s
        m = (j >= 0) & (j < KLEN)
        t[:, s * NP:(s + 1) * NP] = np.where(m, kern[np.clip(j, 0, KLEN - 1)], 0.0)
    return t


# ---------------- device program ----------------

def _build_program():
    nc = bacc.Bacc("TRN2", target_bir_lowering=False, debug=False)

    # all per-core inputs in one tensor -> one DMA -> one DMA-queue semaphore
    # (walrus cannot encode two HW-DGE sem waits on one compute instruction)
    inbuf_d = nc.dram_tensor("inbuf", [NP, INF], F32, kind="ExternalInput")
    out_d = nc.dram_tensor("out", [NP, CH], F32, kind="ExternalOutput")

    with tile.TileContext(nc) as tc:
        with (
            tc.tile_pool(name="main", bufs=1) as mp,
            tc.tile_pool(name="psum", bufs=4, space="PSUM") as pp,
        ):
            # ---- persistent SBUF tiles ----
            INBUF = mp.tile([NP, INF], F32, tag="INBUF")
            X = INBUF[:, 0:CH]
            CONST = INBUF[:, CH:CH + 16]
            CT = INBUF[:, CH + 16:CH + 16 + CH]
            TPRE = INBUF[:, 2 * CH + 16:2 * CH + 16 + NSHIFT * NP]
            TPOST = INBUF[:, 2 * CH + 16 + NSHIFT * NP:2 * CH + 16 + 2 * NSHIFT * NP]
            IDENT = INBUF[:, 2 * CH + 16 + 2 * NSHIFT * NP:INF]
            AX = mp.tile([NP, CH], F32, tag="AX")
            E0 = mp.tile([NP, CH], F32, tag="E0")
            ENV = mp.tile([NP, CH], F32, tag="ENV")
            EINIT = mp.tile([NP, 1], F32, tag="EINIT")
            G1 = mp.tile([NP, CH], F32, tag="G1")
            Y1 = mp.tile([NP, CH], F32, tag="Y1")
            YB = mp.tile([NP, 8, NP + 1], F32, tag="YB")
            Y2C = mp.tile([NP, CH], F32, tag="Y2C")
            At = mp.tile([NP, CH], F32, tag="At")
            Bt = mp.tile([NP, CH], F32, tag="Bt")
            Gt = mp.tile([NP, CH], F32, tag="Gt")
            HP = mp.tile([NP, CH + 1], F32, tag="HP")
            ENDS = mp.tile([NP, 2], F32, tag="ENDS")
            PR = mp.tile([NP, CH], F32, tag="PR")
            PZ = mp.tile([NP, CH], F32, tag="PZ")
            R = mp.tile([NP, CH], F32, tag="R")
            Z = mp.tile([NP, CH], F32, tag="Z")
            M = mp.tile([NP, CH], F32, tag="M")
            U = mp.tile([NP, CH], F32, tag="U")
            SN = mp.tile([NP, CH], F32, tag="SN")
            N = mp.tile([NP, CH], F32, tag="N")
            D = mp.tile([NP, CH], F32, tag="D")
            ZD = mp.tile([NP, CH], F32, tag="ZD")
            Y3 = mp.tile([NP, CH], F32, tag="Y3")
            OUTC = mp.tile([NP, CH], F32, tag="OUTC")

            # const column views
            def cc(i):
                return CONST[:, i:i + 1]
            ONE_MINUS_C, W0, W1, W2, BH2 = cc(0), cc(1), cc(2), cc(3), cc(4)
            WI0, WI1, WI2 = cc(5), cc(6), cc(7)
            BIA, BIB, BI2 = cc(8), cc(9), cc(10)
            NEGSAG, OW, OB = cc(11), cc(12), cc(13)

            # ---- input DMA (single transfer) ----
            nc.sync.dma_start(out=INBUF[:, :], in_=inbuf_d[:, :])

            # ---- zero-init ----
            nc.vector.memset(EINIT[:, :], 0.0)
            nc.vector.memset(HP[:, :], 0.0)
            nc.vector.memset(ENDS[:, :], 0.0)
            nc.vector.memset(YB[:, :, :], 0.0)

            # ---- envelope follower ----
            # AX = (1-c)*|x|
            nc.scalar.activation(AX[:, :], X[:, :], AF.Abs, scale=ONE_MINUS_C)
            nc.vector.tensor_tensor_scan(
                E0[:, :], CT[:, :], AX[:, :], 0.0, OP.mult, OP.add)
            nc.sync.dma_start(out=EINIT[1:NP, 0:1], in_=E0[0:NP - 1, CH - 1:CH])
            nc.vector.tensor_tensor_scan(
                ENV[:, :], CT[:, :], AX[:, :], EINIT[:, 0:1], OP.mult, OP.add)

            # ---- sag gain: y1 = (1 - sag*env) * x ----
            nc.scalar.activation(G1[:, :], ENV[:, :], AF.Identity, scale=NEGSAG, bias=1.0)
            nc.vector.tensor_mul(Y1[:, :], G1[:, :], X[:, :])

            # ---- conv stage helper (chunk-major in -> chunk-major out) ----
            def conv_stage(src, table, dst):
                # chunk -> block transposes
                for v in range(8):
                    pt = pp.tile([NP, NP], F32, tag="pt")
                    nc.tensor.transpose(pt[:, :], src[:, v * NP:(v + 1) * NP], IDENT[:, :])
                    nc.vector.tensor_copy(YB[:, v, 1:NP + 1], pt[:, :])
                # conv: out_chunk[p, 128v+i] = sum_s YB_{v-s}[k, ...] @ T_s[k, i]
                for v in range(8):
                    po = pp.tile([NP, NP], F32, tag="po")
                    for s in range(NSHIFT):
                        w = (v - s) % 8
                        lo = 1 if v - s >= 0 else 0
                        nc.tensor.matmul(
                            po[:, :],
                            YB[:, w, lo:lo + NP],
                            table[:, s * NP:(s + 1) * NP],
                            start=(s == 0),
                            stop=(s == NSHIFT - 1),
                        )
                    nc.scalar.copy(dst[:, v * NP:(v + 1) * NP], po[:, :])

            # ---- pre biquad cascade as FIR ----
            conv_stage(Y1, TPRE, Y2C)

            # ---- GRU gate preactivations from y2 ----
            nc.scalar.activation(At[:, :], Y2C[:, :], AF.Identity, scale=WI0, bias=BIA)
            nc.scalar.activation(Bt[:, :], Y2C[:, :], AF.Identity, scale=WI1, bias=BIB)
            nc.scalar.activation(Gt[:, :], Y2C[:, :], AF.Identity, scale=WI2, bias=BI2)

            # ---- K Jacobi sweeps with double-lagged chunk boundaries ----
            for k in range(K_SWEEPS):
                sl = k % 2
                hp = HP[:, 0:CH]
                # install boundary states (ends of sweep k-2, partition-shifted)
                nc.vector.tensor_copy(HP[:, 0:1], ENDS[:, sl:sl + 1])
                nc.vector.scalar_tensor_tensor(
                    PR[:, :], hp, W0, At[:, :], OP.mult, OP.add)
                nc.vector.scalar_tensor_tensor(
                    PZ[:, :], hp, W1, Bt[:, :], OP.mult, OP.add)
                nc.scalar.activation(R[:, :], PR[:, :], AF.Sigmoid)
                nc.scalar.activation(Z[:, :], PZ[:, :], AF.Sigmoid)
                nc.scalar.activation(M[:, :], hp, AF.Identity, scale=W2, bias=BH2)
                nc.vector.tensor_mul(U[:, :], R[:, :], M[:, :])
                nc.vector.tensor_add(SN[:, :], U[:, :], Gt[:, :])
                nc.scalar.activation(N[:, :], SN[:, :], AF.Tanh)
                nc.vector.tensor_sub(D[:, :], hp, N[:, :])
                nc.vector.tensor_mul(ZD[:, :], Z[:, :], D[:, :])
                nc.vector.tensor_add(HP[:, 1:CH + 1], N[:, :], ZD[:, :])
                # capture chunk ends (partition-shifted) for sweep k+2
                nc.sync.dma_start(out=ENDS[1:NP, sl:sl + 1], in_=HP[0:NP - 1, CH:CH + 1])

            # ---- y3 = ow*h + ob ----
            nc.scalar.activation(Y3[:, :], HP[:, 1:CH + 1], AF.Identity, scale=OW, bias=OB)

            # ---- post cascade + feedback as one FIR ----
            conv_stage(Y3, TPOST, OUTC)

            # ---- output ----
            nc.sync.dma_start(out=out_d[:, :], in_=OUTC[:, :])

    nc.compile()
    return nc


def _get_program():
    if "nc" not in _CACHE:
        _CACHE["nc"] = _build_program()
    return _CACHE["nc"]


# ---------------- entry point ----------------

def prepare_in_maps(inputs):
    x = np.asarray(inputs["x"], dtype=np.float32)
    knobs = np.asarray(inputs["knobs"], dtype=np.float64)
    env_coef_raw = np.asarray(inputs["env_coef_raw"], dtype=np.float64)
    pre_params = np.asarray(inputs["pre_params"], dtype=np.float64)
    post_params = np.asarray(inputs["post_params"], dtype=np.float64)
    gru_wi = np.asarray(inputs["gru_wi"], dtype=np.float64)
    gru_wh = np.asarray(inputs["gru_wh"], dtype=np.float64)
    gru_bi = np.asarray(inputs["gru_bi"], dtype=np.float64)
    gru_bh = np.asarray(inputs["gru_bh"], dtype=np.float64)
    gru_ow = np.asarray(inputs["gru_ow"], dtype=np.float64)
    gru_ob = np.asarray(inputs["gru_ob"], dtype=np.float64)
    sag_w1 = np.asarray(inputs["sag_w1"], dtype=np.float64)
    sag_b1 = np.asarray(inputs["sag_b1"], dtype=np.float64)
    sag_w2 = np.asarray(inputs["sag_w2"], dtype=np.float64)
    sag_b2 = np.asarray(inputs["sag_b2"], dtype=np.float64)
    fb_w1 = np.asarray(inputs["fb_w1"], dtype=np.float64)
    fb_b1 = np.asarray(inputs["fb_b1"], dtype=np.float64)
    fb_w2 = np.asarray(inputs["fb_w2"], dtype=np.float64)
    fb_b2 = np.asarray(inputs["fb_b2"], dtype=np.float64)
    fb_amount = np.asarray(inputs["fb_amount"], dtype=np.float64)

    # tiny host-side parameter math
    c_env = _sigmoid(env_coef_raw[0])
    sag_h = np.tanh(knobs[:, 0:1] @ sag_w1.T + sag_b1)
    sag_depth = _sigmoid(sag_h @ sag_w2.T + sag_b2)[:, 0]          # (B,)
    pre = _stable(pre_params)
    post = _stable(post_params)
    fb_h = np.tanh(knobs[:, 1:3] @ fb_w1.T + fb_b1)
    fb_c = _stable(fb_h @ fb_w2.T + fb_b2)                         # (B,5)
    fb_mix = _sigmoid(fb_amount[0])

    EXT = 2 * KLEN
    kern_pre = np.convolve(_biquad_ir(pre[0], EXT), _biquad_ir(pre[1], EXT))[:KLEN]
    kern_post = np.convolve(_biquad_ir(post[0], EXT), _biquad_ir(post[1], EXT))[:EXT]
    tpre = _toeplitz(kern_pre.astype(np.float32))

    wi0 = gru_wi[:, 0]
    wh0 = gru_wh[:, 0]
    consts_common = [
        1.0 - c_env, wh0[0], wh0[1], wh0[2], gru_bh[2],
        wi0[0], wi0[1], wi0[2],
        gru_bi[0] + gru_bh[0], gru_bi[1] + gru_bh[1], gru_bi[2],
    ]
    crow = np.full((NP, CH), c_env, dtype=np.float32)
    ident = np.eye(NP, dtype=np.float32)

    in_maps = []
    for b in range(B):
        dmix = np.zeros(EXT)
        dmix[0] = 1.0
        dmix -= fb_mix * _biquad_ir(fb_c[b], EXT)
        kern_out = np.convolve(kern_post, dmix)[:KLEN]
        tpost = _toeplitz(kern_out.astype(np.float32))
        consts = np.zeros((NP, 16), dtype=np.float32)
        consts[:, :NCONST] = np.array(
            consts_common + [-sag_depth[b], gru_ow[0], gru_ob[0]],
            dtype=np.float32)[None, :]
        inbuf = np.concatenate([
            x[b].reshape(NP, CH), consts, crow, tpre, tpost, ident,
        ], axis=1).astype(np.float32)
        in_maps.append({"inbuf": np.ascontiguousarray(inbuf)})
    return in_maps


def kernel(**inputs):
    in_maps = prepare_in_maps(inputs)
    global _last_in_maps
    _last_in_maps = in_maps
    nc = _get_program()
    res = run_bass_kernel_spmd(nc, in_maps, list(range(B)))
    out = np.stack([
        np.asarray(res.results[b]["out"], dtype=np.float32).reshape(L)
        for b in range(B)
    ])
    return out
